# revision 1
# baseline (speedup 1.0000x reference)
"""Trainium2 Bass kernel for nn_EncoderLayer_73315091743398.

The reference module's attention einsums ('hwink,hwijm->hwinm') sum their k/j
indices independently, so the whole attention block collapses to, per
(h,w)-chunk c and head i, over the flat q matrix qf = x@Wq.T + pe viewed as
(8192, 512) in raw (s,h,w) row order:

    u[s]  = sum_d qf[c*512+s, 64i+d]          (segment row sums)
    a     = softmax_s(u)
    v[d]  = sum_s a[s] * qf[c*512+s, 64i+d]
    row   = tile8(v) @ Wfc.T = v @ M,  M[d,:] = sum_b Wfc[:, 64b+d].T

and attn_out viewed (S,H,W,D) has row A[s'] = row_{c=s'//32, i=(s'%32)//4},
independent of (h,w).  Core k owns raw rows [k*1024,(k+1)*1024): these are
exactly attention chunks {2k, 2k+1} AND the residual/FFN rows for
s' in [64k, 64k+64), so the 8 cores run fully independent SPMD programs
(data-parallel over the flat row dimension; no collectives).
"""

import math
import os
import sys
from contextlib import ExitStack

import numpy as np
import ml_dtypes  # noqa: F401  (registers bfloat16)

for _p in ("/opt/trn_rl_repo", "/root/.axon_site/_ro/trn_rl_repo"):
    if os.path.isdir(_p) and _p not in sys.path:
        sys.path.append(_p)

import concourse.bass as bass
import concourse.bacc as bacc
import concourse.mybir as mybir
import concourse.tile as tile
from concourse.bass_utils import run_bass_kernel_spmd

F32 = mybir.dt.float32
F32R = mybir.dt.float32r
BF16 = mybir.dt.bfloat16
AF = mybir.ActivationFunctionType
ALU = mybir.AluOpType
AX = mybir.AxisListType

S, H, W, D = 512, 4, 4, 512
NH, DEP, DFF = 8, 64, 2048
NCORES = 8
R = 1024          # rows per core of the flat (8192, 512) view
EPS = 1e-5

# packed fp32 constant block column offsets
O_EYE, O_ON1, O_B2, O_G1, O_BE1, O_G2, O_BE2, O_B1G = (
    0, 128, 129, 641, 1153, 1665, 2177, 2689)
O_G1C, O_BE1C = 2705, 2709
NCF = 2713
# packed f32r constant block column offsets
O_MST, O_ZER, O_ONR, O_E8 = 0, 512, 528, 529
O_EYR, O_B2R = 1041, 1169
NCR = 1681

_cached = {}


def build_nc():
    """Build the single-core SPMD Bass/Tile program (same program on all 8)."""
    nc = bacc.Bacc("TRN2", debug=False, target_bir_lowering=False)

    xT = nc.dram_tensor("xT", [D, R], F32R, kind="ExternalInput")
    xR = nc.dram_tensor("xR", [R, D], F32, kind="ExternalInput")
    peR = nc.dram_tensor("peR", [R, D], F32, kind="ExternalInput")
    WqT = nc.dram_tensor("WqT", [D, D], F32R, kind="ExternalInput")
    W1T = nc.dram_tensor("W1T", [D, DFF], BF16, kind="ExternalInput")
    W2T = nc.dram_tensor("W2T", [DFF, D], BF16, kind="ExternalInput")
    CF = nc.dram_tensor("CF", [128, NCF], F32, kind="ExternalInput")
    CR = nc.dram_tensor("CR", [128, NCR], F32R, kind="ExternalInput")
    CB = nc.dram_tensor("CB", [128, 128 + D], BF16, kind="ExternalInput")
    out = nc.dram_tensor("out", [R, D], F32, kind="ExternalOutput")

    with ExitStack() as ctx:
        tc = ctx.enter_context(tile.TileContext(nc))
        cst = ctx.enter_context(tc.tile_pool(name="cst", bufs=1))
        xp = ctx.enter_context(tc.tile_pool(name="xp", bufs=1))
        qp = ctx.enter_context(tc.tile_pool(name="qp", bufs=1))
        wk = ctx.enter_context(tc.tile_pool(name="wk", bufs=2))
        ps = ctx.enter_context(tc.tile_pool(name="ps", bufs=1, space="PSUM"))

        # ---- loads, cheapest-needed-first so PE can start early ----
        # xq[i] holds m-pair (2i, 2i+1), columns laid out (dt, mi, c)
        xq = [xp.tile([128, R], F32R, tag=f"dT{i}", name=f"xq{i}")
              for i in range(4)]
        wq_all = cst.tile([128, 4 * D], F32R, tag="wq", name="wq_all")
        nc.sync.dma_start(wq_all[:].rearrange("p (t j) -> p t j", t=4),
                          WqT.rearrange("(t p) j -> p t j", p=128))
        pe_sb = [xp.tile([128, D], F32, tag=f"pe{m}", name=f"pesb{m}")
                 for m in range(8)]
        for i in range(4):
            nc.sync.dma_start(xq[i][:], xT[i * 128:(i + 1) * 128, :])
            nc.sync.dma_start(pe_sb[2 * i][:],
                              peR[2 * i * 128:(2 * i + 1) * 128, :])
            nc.sync.dma_start(pe_sb[2 * i + 1][:],
                              peR[(2 * i + 1) * 128:(2 * i + 2) * 128, :])
        cf = cst.tile([128, NCF], F32, tag="cf", name="cf")
        nc.sync.dma_start(cf[:], CF[:])
        cr = cst.tile([128, NCR], F32R, tag="cr", name="cr")
        nc.sync.dma_start(cr[:], CR[:])
        cfb = cst.tile([128, 128 + D], BF16, tag="cfb", name="cfb")
        nc.sync.dma_start(cfb[:], CB[:])
        w1_all = cst.tile([128, 4 * DFF], BF16, tag="w1", name="w1_all")
        nc.sync.dma_start(w1_all[:].rearrange("p (t j) -> p t j", t=4),
                          W1T.rearrange("(t p) j -> p t j", p=128))
        w2_all = cst.tile([128, 16 * D], BF16, tag="w2", name="w2_all")
        nc.sync.dma_start(w2_all[:].rearrange("p (t j) -> p t j", t=16),
                          W2T.rearrange("(t p) j -> p t j", p=128))

        eye_sb = cf[:, O_EYE:O_EYE + 128]
        on1r = cr[:, O_ONR:O_ONR + 1]
        Mst_sb = cr[:, O_MST:O_MST + D]
        zer8 = cr[:, O_ZER:O_ZER + 8]
        B2_sb = cf[:, O_B2:O_B2 + D]
        G1_sb = cf[:, O_G1:O_G1 + D]
        BE1_sb = cf[:, O_BE1:O_BE1 + D]
        G2_sb = cf[:, O_G2:O_G2 + D]
        BE2_sb = cf[:, O_BE2:O_BE2 + D]
        epsT = cst.tile([128, 1], F32, tag="eps", name="epsT")
        nc.vector.memset(epsT[:], EPS)

        q_sb = [qp.tile([128, D], F32, tag=f"q{m}", name=f"qsb{m}") for m in range(8)]
        o1_sb = [qp.tile([128, D], F32, tag=f"o1{m}", name=f"o1sb{m}") for m in range(8)]
        uT = [qp.tile([8, D], F32, tag=f"uT{c}", name=f"uTsb{c}") for c in range(2)]

        def layernorm(dst, zin, g_t, be_t):
            """dst = LN(zin) * g + be for a 128-row tile (zin SBUF f32)."""
            st6 = wk.tile([128, 6], F32, tag="ls")
            nc.vector.bn_stats(st6[:], zin[:])
            mv = wk.tile([128, 2], F32, tag="lm")
            nc.vector.bn_aggr(mv[:], st6[:])
            mu = mv[:, 0:1]
            sd = wk.tile([128, 1], F32, tag="lsd")
            nc.scalar.activation(sd[:], mv[:, 1:2], AF.Sqrt, bias=epsT[:, :])
            rsd = wk.tile([128, 1], F32, tag="lr")
            nc.vector.reciprocal(rsd[:], sd[:])
            nrm = wk.tile([128, D], F32, tag="ln", bufs=1)
            nc.vector.tensor_scalar(nrm[:], zin[:], mu[:], rsd[:],
                                    op0=ALU.subtract, op1=ALU.mult)
            if be_t is None:
                nc.vector.tensor_mul(dst[:], nrm[:], g_t[:])
            else:
                nc.vector.tensor_mul(nrm[:], nrm[:], g_t[:])
                nc.vector.tensor_add(dst[:], nrm[:], be_t[:])

        def q_stage(m):
            qps = ps.tile([128, D], F32, tag="mmA", bufs=2)
            i, mi = divmod(m, 2)
            for dt in range(4):
                nc.tensor.matmul(
                    qps[:],
                    xq[i][:, dt * 256 + mi * 128:dt * 256 + (mi + 1) * 128],
                    wq_all[:, dt * D:(dt + 1) * D],
                    start=(dt == 0), stop=(dt == 3))
            nc.vector.tensor_add(q_sb[m][:], qps[:], pe_sb[m][:])
            useg = wk.tile([128, 8], F32, tag="useg")
            nc.vector.tensor_reduce(
                useg[:], q_sb[m][:].rearrange("p (h d) -> p h d", h=8),
                axis=AX.X, op=ALU.add)
            utp = ps.tile([8, 128], F32, tag="tp", bufs=2)
            nc.tensor.transpose(utp[:], useg[:], eye_sb)
            c, st = divmod(m, 4)
            nc.vector.tensor_copy(uT[c][:, st * 128:(st + 1) * 128], utp[:])

        def attn_softmax(c):
            mx = wk.tile([8, 1], F32, tag="mx")
            nc.vector.tensor_reduce(mx[:], uT[c][:], axis=AX.X, op=ALU.max)
            nmx = wk.tile([8, 1], F32, tag="nmx")
            nc.vector.tensor_scalar_mul(nmx[:], mx[:], -1.0)
            ex = wk.tile([8, D], F32, tag=f"ex{c}", bufs=1)
            ssum = wk.tile([8, 1], F32, tag="esum")
            nc.scalar.activation(ex[:], uT[c][:], AF.Exp, bias=nmx[:, :],
                                 accum_out=ssum[:])
            rcp = wk.tile([8, 1], F32, tag="ercp")
            nc.vector.reciprocal(rcp[:], ssum[:])
            nc.vector.tensor_scalar_mul(ex[:], ex[:], rcp[:])
            return ex

        def attn_prods(c, a_t):
            aTss = []
            for st in range(4):
                atp = ps.tile([128, 8], F32, tag="tp", bufs=2)
                nc.tensor.transpose(atp[:], a_t[:, st * 128:(st + 1) * 128],
                                    eye_sb[:8, :8])
                aTs = wk.tile([128, 8], F32, tag=f"aT{st}", bufs=1)
                nc.vector.tensor_copy(aTs[:], atp[:])
                aTss.append(aTs)
            return aTss

        def attn_la(c, aTss):
            vm = wk.tile([128, 8], F32R, tag="vm")
            nc.vector.tensor_copy(vm[:], zer8)
            for jt in range(4):
                # wsum[p, i] = sum_s q[s, 128*jt+p] * a_i[s]
                wsum = ps.tile([128, 8], F32, tag="vc", bufs=2)
                for st in range(4):
                    nc.tensor.matmul(
                        wsum[:], q_sb[c * 4 + st][:, jt * 128:(jt + 1) * 128],
                        aTss[st][:], start=(st == 0), stop=(st == 3))
                nc.vector.tensor_copy(vm[0:64, 2 * jt:2 * jt + 1],
                                      wsum[0:64, 2 * jt:2 * jt + 1])
                nc.vector.tensor_copy(vm[64:128, 2 * jt + 1:2 * jt + 2],
                                      wsum[64:128, 2 * jt + 1:2 * jt + 2])
            lap = ps.tile([8, D], F32, tag="vc", bufs=2)
            nc.tensor.matmul(lap[:], vm[:], Mst_sb, start=True, stop=True)
            las = wk.tile([8, D], F32R, tag=f"las{c}", bufs=1)
            nc.vector.tensor_copy(las[:], lap[:])
            return las

        def attn_resid(c, las, jt):
            m = c * 4 + jt
            bcp = ps.tile([128, D], F32, tag="mmB", bufs=2)
            nc.tensor.matmul(bcp[:],
                             cr[0:8, O_E8 + jt * 128:O_E8 + (jt + 1) * 128],
                             las[:], start=True, stop=True)
            xrt = wk.tile([128, D], F32, tag="xr")
            nc.gpsimd.dma_start(xrt[:], xR[m * 128:(m + 1) * 128, :])
            z1 = wk.tile([128, D], F32, tag="z1", bufs=1)
            nc.vector.tensor_add(z1[:], bcp[:], xrt[:])
            layernorm(o1_sb[m], z1, G1_sb, None)

        o1T2 = [xp.tile([128, 2 * R], BF16, tag=f"dTh{i}", name=f"o1Th{i}")
                for i in range(2)]

        def trans_stage(m):
            for dt in range(4):
                tps = ps.tile([128, 128], F32, tag="tp", bufs=2)
                nc.tensor.transpose(tps[:], o1_sb[m][:, dt * 128:(dt + 1) * 128],
                                    eye_sb)
                h, mh = divmod(m, 4)
                nc.vector.tensor_scalar(
                    o1T2[h][:, dt * 512 + mh * 128:dt * 512 + (mh + 1) * 128],
                    tps[:], cf[:, O_BE1C + dt:O_BE1C + dt + 1], None,
                    op0=ALU.add)

        h1store = {}

        def ffn_h1(rq):
            h1s = []
            for ft in range(16):
                p1 = ps.tile([128, 256], F32, tag="mmA", bufs=2)
                for dt in range(4):
                    nc.tensor.matmul(
                        p1[:],
                        w1_all[:, dt * DFF + ft * 128:dt * DFF + (ft + 1) * 128],
                        o1T2[rq // 2][:, dt * 512 + (rq % 2) * 256:
                                      dt * 512 + (rq % 2) * 256 + 256],
                        start=(dt == 0), stop=(dt == 3))
                h1 = wk.tile([128, 256], BF16, tag=f"h1_{ft}", bufs=1)
                nc.scalar.activation(h1[:], p1[:], AF.Relu,
                                     bias=cf[:, O_B1G + ft:O_B1G + ft + 1])
                h1s.append(h1)
            h1store[rq] = h1s

        def ffn_rm(rq):
            h1s = h1store[rq]
            for rm in range(2):
                m = rq * 2 + rm
                p2 = ps.tile([128, D], F32, tag="mmB", bufs=2)
                for ft in range(16):
                    nc.tensor.matmul(
                        p2[:], h1s[ft][:, rm * 128:(rm + 1) * 128],
                        w2_all[:, ft * D:(ft + 1) * D],
                        start=(ft == 0), stop=False)
                nc.tensor.matmul(p2[:], cfb[:, 0:128], cfb[:, 128:128 + D],
                                 start=False, stop=True)
                z2 = wk.tile([128, D], F32, tag="z2", bufs=1)
                nc.vector.tensor_add(z2[:], p2[:], o1_sb[m][:])
                yt = wk.tile([128, D], F32, tag="yt", bufs=1)
                layernorm(yt, z2, G2_sb, BE2_sb)
                nc.sync.dma_start(out[m * 128:(m + 1) * 128, :], yt[:])

        for m in range(4):
            q_stage(m)
        a0 = attn_softmax(0)
        q_stage(4)
        aT0 = attn_prods(0, a0)
        q_stage(5)
        q_stage(6)
        las0 = attn_la(0, aT0)
        q_stage(7)
        for jt in range(4):
            attn_resid(0, las0, jt)
        a1 = attn_softmax(1)
        for m in range(4):
            trans_stage(m)
        aT1 = attn_prods(1, a1)
        las1 = attn_la(1, aT1)
        ffn_h1(0)
        attn_resid(1, las1, 0)
        attn_resid(1, las1, 1)
        ffn_rm(0)
        attn_resid(1, las1, 2)
        attn_resid(1, las1, 3)
        ffn_h1(1)
        ffn_rm(1)
        trans_stage(4)
        trans_stage(5)
        ffn_h1(2)
        trans_stage(6)
        trans_stage(7)
        ffn_rm(2)
        ffn_h1(3)
        ffn_rm(3)

    nc.compile()
    return nc


def _round_f32r(a):
    b = np.ascontiguousarray(a, dtype=np.float32).view(np.uint32)
    out = (b + 0x7FF + ((b >> 12) & 1)) & np.uint32(0xFFFFF000)
    return out.view(np.float32)


def _pe_table():
    pos = np.arange(S, dtype=np.float32)[:, None]
    div = np.exp(np.arange(0, D, 2, dtype=np.float32) * (-math.log(10000.0) / D))
    ang = pos * div
    pe = np.zeros((S, D), np.float32)
    pe[:, 0::2] = np.sin(ang)
    pe[:, 1::2] = np.cos(ang)
    return pe


def make_in_maps(x, Wq, Wfc, W1, b1, W2, b2, g1, be1, g2, be2):
    f32 = lambda a: np.ascontiguousarray(a, dtype=np.float32)
    xf = f32(x).reshape(S * H * W, D)
    pe = _pe_table()
    M = f32(Wfc).reshape(D, NH, DEP).sum(axis=1).T          # (64, 512)
    Mstk = np.concatenate([M, M], axis=0)                   # (128, 512)

    CF = np.zeros((128, NCF), np.float32)
    CF[:, O_EYE:O_EYE + 128] = np.eye(128, dtype=np.float32)
    CF[:, O_ON1] = 1.0
    CF[:, O_B2:O_B2 + D] = np.tile(f32(b2), (128, 1))
    CF[:, O_G1:O_G1 + D] = np.tile(f32(g1), (128, 1))
    CF[:, O_BE1:O_BE1 + D] = np.tile(f32(be1), (128, 1))
    CF[:, O_G2:O_G2 + D] = np.tile(f32(g2), (128, 1))
    CF[:, O_BE2:O_BE2 + D] = np.tile(f32(be2), (128, 1))
    CF[:, O_B1G:O_B1G + 16] = f32(b1).reshape(16, 128).T
    CF[:, O_G1C:O_G1C + 4] = f32(g1).reshape(4, 128).T
    CF[:, O_BE1C:O_BE1C + 4] = f32(be1).reshape(4, 128).T

    CB = np.zeros((128, 128 + D), np.float32)
    CB[:, 0:128] = np.eye(128, dtype=np.float32)
    CB[:, 128:128 + D] = np.tile(f32(b2) + f32(be1), (128, 1))
    CB = np.asarray(CB, dtype="bfloat16")
    CR = np.zeros((128, NCR), np.float32)
    CR[:, O_MST:O_MST + D] = _round_f32r(Mstk)
    CR[:, O_ONR] = 1.0
    CR[:, O_EYR:O_EYR + 128] = np.eye(128, dtype=np.float32)
    CR[:, O_B2R:O_B2R + D] = _round_f32r(np.tile(f32(b2) + f32(be1), (128, 1)))
    for jt in range(4):
        for p in range(128):
            CR[2 * jt + p // 64, O_E8 + jt * 128 + p] = 1.0

    shared = dict(
        WqT=_round_f32r(Wq.T),
        W1T=np.asarray(f32(W1.T), dtype='bfloat16'),
        W2T=np.asarray(f32(W2.T), dtype='bfloat16'),
        CF=CF, CR=CR, CB=CB,
    )
    maps = []
    for k in range(NCORES):
        sl = xf[k * R:(k + 1) * R]
        m = dict(shared)
        slT = _round_f32r(sl.T)
        # xq layout: row-block i = m-pair (2i, 2i+1); columns (dt, mi, c)
        arr = slT.reshape(4, 128, 4, 2, 128)        # (t, p, i, mi, c)
        arr = arr.transpose(2, 1, 0, 3, 4)          # (i, p, t, mi, c)
        m["xT"] = np.ascontiguousarray(arr.reshape(512, 1024))
        m["xR"] = np.ascontiguousarray(sl)
        m["peR"] = np.ascontiguousarray(np.repeat(pe[k * 64:(k + 1) * 64], 16, axis=0))
        maps.append(m)
    return maps


def kernel(x, Wq, Wfc, W1, b1, W2, b2, g1, be1, g2, be2, _results_hook=None,
           _trace=False, _tmpdir=None):
    if "nc" not in _cached:
        _cached["nc"] = build_nc()
    nc = _cached["nc"]
    in_maps = make_in_maps(x, Wq, Wfc, W1, b1, W2, b2, g1, be1, g2, be2)
    res = run_bass_kernel_spmd(nc, in_maps, list(range(NCORES)),
                               trace=_trace, tmpdir=_tmpdir)
    if _results_hook is not None:
        _results_hook(res)
    y = np.concatenate([res.results[k]["out"] for k in range(NCORES)], axis=0)
    return y.reshape(S, H, W, D)



# revision 11
# speedup vs baseline: 1.0794x; 1.0794x over previous
"""Trainium2 Bass kernel for nn_EncoderLayer_73315091743398.

The reference attention einsums ('hwink,hwijm->hwinm') sum their k/j indices
independently, so per (h,w)-chunk c and head i, over the flat matrix
xf = x viewed (8192, 512) in raw (s,h,w) row order with qf = xf@Wq.T + pe:

    u[s]  = sum_{d in seg_i} qf[c*512+s, d]        (segment row sums)
    a     = softmax_s(u)
    v[d]  = sum_s a[s] * qf[c*512+s, 64i+d]
    row   = tile8(v) @ Wfc.T = v @ M,   M[d,:] = sum_b Wfc[:, 64b+d].T

and attn_out viewed (S,H,W,D) has row A[s'] = row_{c=s'//32, i=(s'%32)//4}.
Unlike the previous version, q = x@Wq.T is never materialized:
    u = X_c @ wseg + pu          (wseg = per-head column sums of Wq.T)
    y = a @ X_c ;  G = y @ Wq.T + ape @ pe_c ;  v_i = G[i, seg_i]
which removes the full q GEMM and its transposed-weight load entirely.

Biases fold away: g1 into W1 (host), be1/b1 into the relu bias (host),
b2/be1 vanish from the second LayerNorm by shift invariance.

Core k owns flat rows [k*1024,(k+1)*1024) = attention chunks {2k, 2k+1};
8 cores run fully independent SPMD programs (no collectives).
"""

import math
import os
import sys
from contextlib import ExitStack

import numpy as np
import ml_dtypes  # noqa: F401  (registers bfloat16)

for _p in ("/opt/trn_rl_repo", "/root/.axon_site/_ro/trn_rl_repo"):
    if os.path.isdir(_p) and _p not in sys.path:
        sys.path.append(_p)

import concourse.bass as bass  # noqa: F401
import concourse.bacc as bacc
import concourse.mybir as mybir
import concourse.tile as tile
from concourse.bass_utils import run_bass_kernel_spmd

F32 = mybir.dt.float32
F32R = mybir.dt.float32r
BF16 = mybir.dt.bfloat16
AF = mybir.ActivationFunctionType
ALU = mybir.AluOpType
AX = mybir.AxisListType

S, H, W, D = 512, 4, 4, 512
NH, DEP, DFF = 8, 64, 2048
NCORES = 8
R = 1024          # rows per core of the flat (8192, 512) view
EPS = 1e-5

_cached = {}


def build_nc(flags):
    """Build the single-core SPMD Bass/Tile program (same program on all 8).

    flags = (need_g1, need_g2, need_be2): include general affine ops only
    when the corresponding parameter is nontrivial.
    """
    need_g1, need_g2, need_be2 = flags
    nc = bacc.Bacc("TRN2", debug=False, target_bir_lowering=False)

    # ---- dram tensors (host pre-laid-out to match SBUF tiles 1:1) ----
    d_wsegT = nc.dram_tensor("wsegT", [128, 32], F32R, kind="ExternalInput")
    d_REP = nc.dram_tensor("REP", [32, 512], F32R, kind="ExternalInput")
    d_puT = nc.dram_tensor("puT", [32, 16], F32R, kind="ExternalInput")
    d_eyeS = nc.dram_tensor("eyeS", [8, 8], F32, kind="ExternalInput")
    d_eye128 = nc.dram_tensor("eye128", [128, 128], F32, kind="ExternalInput")
    d_eye128r = nc.dram_tensor("eye128r", [128, 128], F32R, kind="ExternalInput")
    d_peC = nc.dram_tensor("peC", [32, 1024], F32R, kind="ExternalInput")
    d_E8 = nc.dram_tensor("E8", [8, 512], F32R, kind="ExternalInput")
    d_Mst = nc.dram_tensor("Mst", [128, 512], F32R, kind="ExternalInput")
    d_b1g = nc.dram_tensor("b1g", [128, 16], F32, kind="ExternalInput")
    d_xT = [nc.dram_tensor(f"xT{c}_{jb}", [128, 512], F32R, kind="ExternalInput")
            for c in range(2) for jb in range(4)]
    d_xR = [nc.dram_tensor(f"xR{m}", [128, 512], F32R, kind="ExternalInput")
            for m in range(8)]
    d_wq = [nc.dram_tensor(f"wqT{jb}", [128, 512], BF16, kind="ExternalInput")
            for jb in range(4)]
    d_w1 = [nc.dram_tensor(f"w1t{dt}", [128, 2048], BF16, kind="ExternalInput")
            for dt in range(4)]
    d_w2 = [nc.dram_tensor(f"w2t{g}", [128, 2048], BF16, kind="ExternalInput")
            for g in range(4)]
    if need_g1:
        d_g1 = nc.dram_tensor("g1R", [128, 512], F32, kind="ExternalInput")
    if need_g2:
        d_g2 = nc.dram_tensor("g2R", [128, 512], F32, kind="ExternalInput")
    if need_be2:
        d_be2 = nc.dram_tensor("be2R", [128, 512], F32, kind="ExternalInput")
    d_out = nc.dram_tensor("out", [R, D], F32, kind="ExternalOutput")

    with ExitStack() as ctx:
        tc = ctx.enter_context(tile.TileContext(nc))
        cst = ctx.enter_context(tc.tile_pool(name="cst", bufs=1))
        xp = ctx.enter_context(tc.tile_pool(name="xp", bufs=1))
        qp = ctx.enter_context(tc.tile_pool(name="qp", bufs=1))
        wk = ctx.enter_context(tc.tile_pool(name="wk", bufs=2))
        ps = ctx.enter_context(tc.tile_pool(name="ps", bufs=1, space="PSUM"))

        # ---- SBUF tiles + loads in DMA priority order ----
        # 1. tiny consts feeding the u-stage
        wsegT = cst.tile([128, 32], F32R, tag="wsegT", name="wsegT")
        nc.sync.dma_start(wsegT[:], d_wsegT[:])
        REP = cst.tile([32, 512], F32R, tag="REP", name="REP")
        nc.sync.dma_start(REP[:], d_REP[:])
        puT = cst.tile([32, 16], F32R, tag="puT", name="puT")
        nc.sync.dma_start(puT[:], d_puT[:])
        eyeS = cst.tile([8, 8], F32, tag="eyeS", name="eyeS")
        nc.sync.dma_start(eyeS[:], d_eyeS[:])

        # 2. chunk-0 activations
        xTt = [[xp.tile([128, 512], F32R, tag=f"xT{c}_{jb}", name=f"xT{c}_{jb}")
                for jb in range(4)] for c in range(2)]
        for jb in range(4):
            nc.sync.dma_start(xTt[0][jb][:], d_xT[jb][:])
        xRt = [xp.tile([128, 512], F32R, tag=f"xR{m}", name=f"xR{m}")
               for m in range(8)]
        for m in range(4):
            nc.sync.dma_start(xRt[m][:], d_xR[m][:])

        # 3. attention tail consts, then chunk-0 FFN weights
        E8 = cst.tile([8, 512], F32R, tag="E8", name="E8")
        nc.sync.dma_start(E8[:], d_E8[:])
        Mst = cst.tile([128, 512], F32R, tag="Mst", name="Mst")
        nc.sync.dma_start(Mst[:], d_Mst[:])
        eye128r = cst.tile([128, 128], F32R, tag="eye128r", name="eye128r")
        nc.sync.dma_start(eye128r[:], d_eye128r[:])
        peC = cst.tile([32, 1024], F32R, tag="peC", name="peC")
        nc.sync.dma_start(peC[:], d_peC[:])
        wqT = [cst.tile([128, 512], BF16, tag=f"wq{jb}", name=f"wq{jb}")
               for jb in range(4)]
        for jb in range(4):
            nc.sync.dma_start(wqT[jb][:], d_wq[jb][:])
        eye128 = cst.tile([128, 128], F32, tag="eye128", name="eye128")
        nc.sync.dma_start(eye128[:], d_eye128[:])
        b1g = cst.tile([128, 16], F32, tag="b1g", name="b1g")
        nc.sync.dma_start(b1g[:], d_b1g[:])
        w1t = [cst.tile([128, 2048], BF16, tag=f"w1t{dt}", name=f"w1t{dt}")
               for dt in range(4)]
        for dt in range(4):
            nc.sync.dma_start(w1t[dt][:], d_w1[dt][:])

        # 4. chunk-1 activations, then second FFN weights
        for jb in range(4):
            nc.sync.dma_start(xTt[1][jb][:], d_xT[4 + jb][:])
        for m in range(4, 8):
            nc.sync.dma_start(xRt[m][:], d_xR[m][:])

        w2t = [cst.tile([128, 2048], BF16, tag=f"w2t{g}", name=f"w2t{g}")
               for g in range(4)]
        for g in range(4):
            nc.sync.dma_start(w2t[g][:], d_w2[g][:])
        if need_g1:
            g1R = cst.tile([128, 512], F32, tag="g1R", name="g1R")
            nc.sync.dma_start(g1R[:], d_g1[:])
        if need_g2:
            g2R = cst.tile([128, 512], F32, tag="g2R", name="g2R")
            nc.sync.dma_start(g2R[:], d_g2[:])
        if need_be2:
            be2R = cst.tile([128, 512], F32, tag="be2R", name="be2R")
            nc.sync.dma_start(be2R[:], d_be2[:])

        epsT = cst.tile([128, 1], F32, tag="eps", name="epsT")
        nc.vector.memset(epsT[:], EPS)
        zer8 = cst.tile([128, 8], F32, tag="zer8", name="zer8")
        nc.vector.memset(zer8[:], 0.0)

        # persistent per-core activations
        o1T = [xp.tile([128, 4 * 512], BF16, tag=f"o1T{c}", name=f"o1T{c}")
               for c in range(2)]
        h1 = [xp.tile([128, 16 * 512], BF16, tag=f"h1_{c}", name=f"h1_{c}")
              for c in range(2)]
        nrm1 = [qp.tile([128, 512], F32, tag=f"nrm{m}", name=f"nrm{m}")
                for m in range(8)]
        lassb = [qp.tile([8, 512], F32R, tag=f"las{c}", name=f"las{c}")
                 for c in range(2)]

        # ---------------- attention ----------------
        def u_stage(c):
            ups = ps.tile([8, 512], F32, tag="sm", bufs=1)
            for jb in range(4):
                nc.tensor.matmul(ups[:], wsegT[:, jb * 8:(jb + 1) * 8],
                                 xTt[c][jb][:], start=(jb == 0), stop=False)
            nc.tensor.matmul(ups[:], puT[:, c * 8:(c + 1) * 8], REP[:],
                             start=False, stop=True)
            return ups

        def softmax(c, ups):
            mx = wk.tile([8, 1], F32, tag="mx")
            nc.vector.tensor_reduce(mx[:], ups[:], axis=AX.X, op=ALU.max)
            nmx = wk.tile([8, 1], F32, tag="nmx")
            nc.vector.tensor_scalar_mul(nmx[:], mx[:], -1.0)
            ex = wk.tile([8, 512], F32, tag=f"ex{c}", bufs=1)
            ssum = wk.tile([8, 1], F32, tag="esum")
            nc.scalar.activation(ex[:], ups[:], AF.Exp, bias=nmx[:, :],
                                 accum_out=ssum[:])
            rcp = wk.tile([8, 1], F32, tag="ercp")
            nc.vector.reciprocal(rcp[:], ssum[:])
            a_sb = wk.tile([8, 512], F32, tag=f"a{c}", bufs=1)
            nc.vector.tensor_scalar_mul(a_sb[:], ex[:], rcp[:])
            ape = wk.tile([8, 32], F32, tag=f"ape{c}", bufs=1)
            nc.vector.tensor_reduce(
                ape[:], a_sb[:].rearrange("p (t u) -> p t u", t=32),
                axis=AX.X, op=ALU.add)
            return a_sb, ape

        def attn_core(c, a_sb, ape):
            # aT: [s, i] blocks for the Y matmuls
            aTs = []
            for sb in range(4):
                atp = ps.tile([128, 8], F32, tag="tp", bufs=2)
                nc.tensor.transpose(atp[:], a_sb[:, sb * 128:(sb + 1) * 128],
                                    eyeS[:])
                t = wk.tile([128, 8], F32R, tag=f"aT{sb}", bufs=1)
                nc.vector.tensor_copy(t[:], atp[:])
                aTs.append(t)
            aptp = ps.tile([32, 8], F32, tag="tp", bufs=2)
            nc.tensor.transpose(aptp[:], ape[:], eyeS[:])
            apeT = wk.tile([32, 8], F32R, tag="apeT", bufs=1)
            nc.vector.tensor_copy(apeT[:], aptp[:])
            # Y = a @ X  (j-space row aggregate), [8, 512]
            yps = ps.tile([8, 512], F32, tag="sm", bufs=1)
            for sb in range(4):
                nc.tensor.matmul(yps[:], aTs[sb][:], xRt[c * 4 + sb][:],
                                 start=(sb == 0), stop=(sb == 3))
            ysb = wk.tile([8, 512], F32, tag="ysb", bufs=1)
            nc.vector.tensor_copy(ysb[:], yps[:])
            yT = []
            for jb in range(4):
                ytp = ps.tile([128, 8], F32, tag="tp", bufs=2)
                nc.tensor.transpose(ytp[:], ysb[:, jb * 128:(jb + 1) * 128],
                                    eyeS[:])
                t = wk.tile([128, 8], BF16, tag=f"yT{jb}", bufs=1)
                nc.vector.tensor_copy(t[:], ytp[:])
                yT.append(t)
            # G = Y @ Wq.T + ape @ pe_c   [8, 512]
            gps = ps.tile([8, 512], F32, tag="sm", bufs=1)
            for jb in range(4):
                nc.tensor.matmul(gps[:], yT[jb][:], wqT[jb][:],
                                 start=(jb == 0), stop=False)
            nc.tensor.matmul(gps[:], apeT[:], peC[:, c * 512:(c + 1) * 512],
                             start=False, stop=True)
            gsb = wk.tile([8, 512], F32, tag="gsb", bufs=1)
            nc.vector.tensor_copy(gsb[:], gps[:])
            # per-head segment select into vm [128, 8]
            vm = wk.tile([128, 8], F32R, tag=f"vm{c}", bufs=1)
            nc.vector.tensor_copy(vm[:], zer8[:])
            for jb in range(4):
                gtp = ps.tile([128, 8], F32, tag="tp", bufs=2)
                nc.tensor.transpose(gtp[:], gsb[:, jb * 128:(jb + 1) * 128],
                                    eyeS[:])
                nc.vector.tensor_copy(vm[0:64, 2 * jb:2 * jb + 1],
                                      gtp[0:64, 2 * jb:2 * jb + 1])
                nc.vector.tensor_copy(vm[64:128, 2 * jb + 1:2 * jb + 2],
                                      gtp[64:128, 2 * jb + 1:2 * jb + 2])
            # las [8, 512] = attention output rows for the 8 heads
            lps = ps.tile([8, 512], F32, tag="sm", bufs=1)
            nc.tensor.matmul(lps[:], vm[:], Mst[:], start=True, stop=True)
            nc.vector.tensor_copy(lassb[c][:], lps[:])

        def resid_ln1(c, jt):
            """z1 = broadcast(las) + x (both on PE); nrm1 = LN(z1), m = c*4+jt."""
            m = c * 4 + jt
            bcp = ps.tile([128, 512], F32, tag="mm", bufs=3)
            nc.tensor.matmul(bcp[:], E8[:, jt * 128:(jt + 1) * 128],
                             lassb[c][:], start=True, stop=False)
            nc.tensor.matmul(bcp[:], eye128r[:], xRt[m][:],
                             start=False, stop=True)
            st6 = wk.tile([128, 6], F32, tag="ls")
            nc.vector.bn_stats(st6[:], bcp[:])
            mv = wk.tile([128, 2], F32, tag="lm")
            nc.vector.bn_aggr(mv[:], st6[:])
            sd = wk.tile([128, 1], F32, tag="lsd")
            nc.scalar.activation(sd[:], mv[:, 1:2], AF.Sqrt, bias=epsT[:, :])
            rsd = wk.tile([128, 1], F32, tag="lr")
            nc.vector.reciprocal(rsd[:], sd[:])
            nc.vector.tensor_scalar(nrm1[m][:], bcp[:], mv[:, 0:1], rsd[:],
                                    op0=ALU.subtract, op1=ALU.mult)
            if need_g1:
                nc.vector.tensor_mul(nrm1[m][:], nrm1[m][:], g1R[:])

        def trans_stage(m):
            c, jt = divmod(m, 4)
            for dt in range(4):
                tps = ps.tile([128, 128], F32, tag="tp", bufs=2)
                nc.tensor.transpose(tps[:], nrm1[m][:, dt * 128:(dt + 1) * 128],
                                    eye128[:])
                nc.scalar.copy(o1T[c][:, dt * 512 + jt * 128:
                                      dt * 512 + (jt + 1) * 128], tps[:])

        # ---------------- FFN ----------------
        def mm1(c, ft):
            hps = ps.tile([128, 512], F32, tag="mmh", bufs=2)
            for dt in range(4):
                nc.tensor.matmul(
                    hps[:], w1t[dt][:, ft * 128:(ft + 1) * 128],
                    o1T[c][:, dt * 512:(dt + 1) * 512],
                    start=(dt == 0), stop=(dt == 3))
            nc.scalar.activation(h1[c][:, ft * 512:(ft + 1) * 512], hps[:],
                                 AF.Relu, bias=b1g[:, ft:ft + 1])

        def mm2_ln2(c, sb):
            """s-block c*4+sb: p2 = h1.T @ W2.T ; out = LN(nrm1 + p2)."""
            m = c * 4 + sb
            p2 = ps.tile([128, 512], F32, tag="mm", bufs=3)
            for ft in range(16):
                nc.tensor.matmul(
                    p2[:], h1[c][:, ft * 512 + sb * 128:ft * 512 + (sb + 1) * 128],
                    w2t[ft // 4][:, (ft % 4) * 512:(ft % 4 + 1) * 512],
                    start=(ft == 0), stop=(ft == 15))
            z2 = wk.tile([128, 512], F32, tag="z2", bufs=2)
            nc.vector.tensor_add(z2[:], p2[:], nrm1[m][:])
            st6 = wk.tile([128, 6], F32, tag="ls2")
            nc.vector.bn_stats(st6[:], z2[:])
            mv = wk.tile([128, 2], F32, tag="lm2")
            nc.vector.bn_aggr(mv[:], st6[:])
            sd = wk.tile([128, 1], F32, tag="lsd2")
            nc.scalar.activation(sd[:], mv[:, 1:2], AF.Sqrt, bias=epsT[:, :])
            rsd = wk.tile([128, 1], F32, tag="lr2")
            nc.vector.reciprocal(rsd[:], sd[:])
            yt = wk.tile([128, 512], F32, tag="yt", bufs=2)
            nc.vector.tensor_scalar(yt[:], z2[:], mv[:, 0:1], rsd[:],
                                    op0=ALU.subtract, op1=ALU.mult)
            if need_g2:
                nc.vector.tensor_mul(yt[:], yt[:], g2R[:])
            if need_be2:
                nc.vector.tensor_add(yt[:], yt[:], be2R[:])
            nc.sync.dma_start(d_out[m * 128:(m + 1) * 128, :], yt[:])

        # ---------------- schedule ----------------
        u0 = u_stage(0)
        a0, ape0 = softmax(0, u0)
        attn_core(0, a0, ape0)
        for jt in range(4):
            resid_ln1(0, jt)
        for m in range(4):
            trans_stage(m)
        for ft in range(16):
            mm1(0, ft)
        u1 = u_stage(1)
        a1, ape1 = softmax(1, u1)
        attn_core(1, a1, ape1)
        for jt in range(4):
            resid_ln1(1, jt)
        for sb in range(4):
            mm2_ln2(0, sb)
        for m in range(4, 8):
            trans_stage(m)
        for ft in range(16):
            mm1(1, ft)
        for sb in range(4):
            mm2_ln2(1, sb)

    nc.compile()
    return nc


def _round_f32r(a):
    b = np.ascontiguousarray(a, dtype=np.float32).view(np.uint32)
    out = (b + 0x7FF + ((b >> 12) & 1)) & np.uint32(0xFFFFF000)
    return out.view(np.float32)


def _pe_table():
    pos = np.arange(S, dtype=np.float32)[:, None]
    div = np.exp(np.arange(0, D, 2, dtype=np.float32) * (-math.log(10000.0) / D))
    ang = pos * div
    pe = np.zeros((S, D), np.float32)
    pe[:, 0::2] = np.sin(ang)
    pe[:, 1::2] = np.cos(ang)
    return pe


def make_in_maps(x, Wq, Wfc, W1, b1, W2, b2, g1, be1, g2, be2):
    f32 = lambda a: np.ascontiguousarray(a, dtype=np.float32)
    bf16 = lambda a: np.ascontiguousarray(np.asarray(a, dtype=np.float32),
                                          ).astype(ml_dtypes.bfloat16)
    x, Wq, Wfc, W1, W2 = f32(x), f32(Wq), f32(Wfc), f32(W1), f32(W2)
    b1, b2, g1, be1, g2, be2 = map(f32, (b1, b2, g1, be1, g2, be2))
    xf = x.reshape(S * H * W, D)
    pe = _pe_table()
    pe_seg = pe.reshape(S, NH, DEP).sum(-1)              # [s, i]
    wseg = Wq.reshape(NH, DEP, D).sum(1)                 # [i, j]
    M = Wfc.reshape(D, NH, DEP).sum(1).T                 # [64, 512]
    b1t = b1 + W1 @ be1

    need_g1 = bool(np.any(g1 != 1.0))
    need_g2 = bool(np.any(g2 != 1.0))
    need_be2 = bool(np.any(be2 != 0.0))
    flags = (need_g1, need_g2, need_be2)
    # g1 is folded into W1 unless nrm1 is already scaled by g1 on device
    W1g = W1 if need_g1 else W1 * g1[None, :]

    p = np.arange(128)
    shared = {
        "wsegT": _round_f32r(wseg.T.reshape(4, 128, NH).transpose(1, 0, 2)
                             .reshape(128, 32)),
        "REP": _round_f32r((np.arange(512)[None, :] // 16
                            == np.arange(32)[:, None]).astype(np.float32)),
        "eyeS": np.eye(8, dtype=np.float32),
        "eye128": np.eye(128, dtype=np.float32),
        "eye128r": np.eye(128, dtype=np.float32),
        "Mst": _round_f32r(np.concatenate([M, M], axis=0)),
        "b1g": b1t.reshape(16, 128).T.copy(),
    }
    # E8[i, jt*128+p] = 1 iff i == 2*jt + p//64
    E8 = np.zeros((8, 512), np.float32)
    for jt in range(4):
        E8[2 * jt, jt * 128 + np.arange(64)] = 1.0
        E8[2 * jt + 1, jt * 128 + 64 + np.arange(64)] = 1.0
    shared["E8"] = _round_f32r(E8)
    for jb in range(4):
        shared[f"wqT{jb}"] = bf16(Wq[:, jb * 128:(jb + 1) * 128].T)
    for dt in range(4):
        # w1t[dt][p, ft*128+f] = W1g[ft*128+f, dt*128+p]
        shared[f"w1t{dt}"] = bf16(W1g[:, dt * 128:(dt + 1) * 128].T)
    for g in range(4):
        # w2t[g][p, q*512+e] = W2[e, (4g+q)*128+p]
        blk = W2[:, g * 512:(g + 1) * 512].T.reshape(4, 128, D)
        shared[f"w2t{g}"] = bf16(blk.transpose(1, 0, 2).reshape(128, 2048))
    if need_g1:
        shared["g1R"] = np.tile(g1, (128, 1))
    if need_g2:
        shared["g2R"] = np.tile(g2, (128, 1))
    if need_be2:
        shared["be2R"] = np.tile(be2, (128, 1))

    maps = []
    for k in range(NCORES):
        sl = xf[k * R:(k + 1) * R]
        m = dict(shared)
        for c in range(2):
            ch = sl[c * 512:(c + 1) * 512]          # [s, j]
            for jb in range(4):
                m[f"xT{c}_{jb}"] = _round_f32r(ch[:, jb * 128:(jb + 1) * 128].T)
        for mi in range(8):
            m[f"xR{mi}"] = _round_f32r(sl[mi * 128:(mi + 1) * 128])
        crow = 2 * k * 32
        m["puT"] = _round_f32r(
            pe_seg[crow:crow + 64].reshape(2, 32, NH).transpose(1, 0, 2)
            .reshape(32, 16))
        m["peC"] = _round_f32r(
            pe[crow:crow + 64].reshape(2, 32, D).transpose(1, 0, 2)
            .reshape(32, 1024))
        maps.append(m)
    return maps, flags


def kernel(x, Wq, Wfc, W1, b1, W2, b2, g1, be1, g2, be2, _results_hook=None,
           _trace=False, _tmpdir=None):
    in_maps, flags = make_in_maps(x, Wq, Wfc, W1, b1, W2, b2, g1, be1, g2, be2)
    if flags not in _cached:
        _cached[flags] = build_nc(flags)
    nc = _cached[flags]
    res = run_bass_kernel_spmd(nc, in_maps, list(range(NCORES)),
                               trace=_trace, tmpdir=_tmpdir)
    if _results_hook is not None:
        _results_hook(res)
    y = np.concatenate([res.results[k]["out"] for k in range(NCORES)], axis=0)
    return y.reshape(S, H, W, D)
